# revision 15
# baseline (speedup 1.0000x reference)
"""Causal single-head attention (B=8, T=4096, C=1024, H=128) on 8 TRN2
NeuronCores, data-parallel over batch: core b computes batch element b.

Host pre-transposes and pre-casts: each core gets xT [C, T] fp16 (so the
contraction dim is already on partitions -- no on-chip transposes) plus
Wq/Wk/Wv [C, H] fp16 replicated. Output is [T, H] f32.

Per chunk of 512 queries (fully unrolled Tile program, fp16 compute /
fp32 psum):
  phase A: one DMA brings the xT chunk in; PE projects qT/kT [H, 512]
  (contraction C on partitions, N=512 streams) and v [t, H] blocks.
  phase B: key blocks are processed in PAIRS -- two N=512 score matmuls
  land side by side in one 2-bank PSUM tile [128, 1024] and a single
  ScalarE exp (scale fused, fp16 out) covers both, halving the per-call
  ACTIVATE overhead that bottlenecked the per-j version. Diagonal blocks
  compute only the causally needed score columns; the junk exp() left of
  the diagonal is overwritten by a DVE memset, and the diagonal block is
  masked by a triangular multiply. AV accumulates out[q, H+1] over key
  blocks (ones column of v yields softmax denominators); DVE normalizes.
"""
import numpy as np

import concourse.bass as bass
import concourse.mybir as mybir
import concourse.tile as tile
from concourse.bass import ts
from contextlib import ExitStack

F16 = mybir.dt.float16
F32 = mybir.dt.float32

B, T, C, H = 8, 4096, 1024, 128

# ---------------------------------------------------------------------------
# Workaround for the walrus build in this container: each TPB instruction may
# carry at most ONE sync-wait ("Too many sync wait commands" otherwise), but
# Tile attaches several. Keep only the last wait per instruction and hoist the
# others onto preceding same-engine NoOps (engines execute their stream in
# order, so the gating semantics are identical). The tail drain gets the same
# treatment.
# ---------------------------------------------------------------------------
_MAX_WAITS = 1
_orig_add_instruction = tile.TileContext._add_instruction


def _split_waits_add_instruction(self, inst):
    si = inst.sync_info
    if (
        si is not None
        and len(si.on_wait) > _MAX_WAITS
        and inst.engine != mybir.EngineType.Unassigned
    ):
        waits = list(si.on_wait)
        extra, keep = waits[:-_MAX_WAITS], waits[-_MAX_WAITS:]
        for w in extra:
            nop = mybir.InstNoOp(
                name=self.nc.get_next_instruction_name(),
                engine=inst.engine,
                ins=[],
                outs=[],
                bass_nofuse=True,
                sync_info=mybir.SyncInfo(on_wait=[w], on_update=[]),
                debug=inst.debug,
            )
            _orig_add_instruction(self, nop)
        inst.sync_info = mybir.SyncInfo(on_wait=keep, on_update=list(si.on_update))
    return _orig_add_instruction(self, inst)


def _split_drain_and_barrier(self, tick_clock, wait_clock):
    nc = self.nc
    probe = nc.sync.nop(nofuse=True, hint="tile_drain_wait_split")
    wait_clock.add_sem_waits(
        probe.ins, tile.ScopedClock({None: tick_clock.global_clock})
    )
    si = probe.ins.sync_info
    waits = list(si.on_wait) if si is not None else []
    if len(waits) > _MAX_WAITS:
        probe.ins.sync_info = mybir.SyncInfo(
            on_wait=waits[:_MAX_WAITS], on_update=list(si.on_update)
        )
        rest = waits[_MAX_WAITS:]
        for i in range(0, len(rest), _MAX_WAITS):
            extra = nc.sync.nop(nofuse=True, hint=f"tile_drain_wait_split_{i}")
            extra.ins.sync_info = mybir.SyncInfo(
                on_wait=rest[i : i + _MAX_WAITS], on_update=[]
            )
    nc.sync.drain()
    nc.all_engine_barrier()
    assert self.sems is not None
    popped = nc._tile_sem_poison_stack.pop()
    assert popped is self._sem_poison
    nc.clear_and_free_semaphores(list(self.sems.allocated().values()))
    nc.all_engine_barrier()


def _apply_tile_patch():
    tile.TileContext._drain_and_barrier = _split_drain_and_barrier
    tile.TileContext._add_instruction = _split_waits_add_instruction


# ---------------------------------------------------------------------------
# Kernel builder
# ---------------------------------------------------------------------------
def build_attention(dtype=F16):
    TB = T // 128
    CB = C // 128
    NCH = T // 512
    scale = float(H) ** -0.5

    nc = bass.Bass()
    xt = nc.dram_tensor("xt", [C, T], F16, kind="ExternalInput")
    wq = nc.dram_tensor("wq", [C, H], F16, kind="ExternalInput")
    wk = nc.dram_tensor("wk", [C, H], F16, kind="ExternalInput")
    wv = nc.dram_tensor("wv", [C, H], F16, kind="ExternalInput")
    out = nc.dram_tensor("out", [T, H], F32, kind="ExternalOutput")

    with tile.TileContext(nc) as tc, ExitStack() as ctx:
        const = ctx.enter_context(tc.tile_pool(name="const", bufs=1))
        xsb = ctx.enter_context(tc.tile_pool(name="xsb", bufs=3))
        persist = ctx.enter_context(tc.tile_pool(name="persist", bufs=1))
        pP = ctx.enter_context(tc.tile_pool(name="pP", bufs=18))
        osb = ctx.enter_context(tc.tile_pool(name="osb", bufs=4))
        # scores AND projections share one 3-deep pool of 2-bank tiles;
        # ops take the remaining 2 banks (8 total)
        pp = ctx.enter_context(tc.tile_pool(name="pp", bufs=3, space="PSUM"))
        po = ctx.enter_context(tc.tile_pool(name="po", bufs=1, space="PSUM"))

        # weights (replicated, fp16 from host) on the idle Scalar HWDGE
        # queue: one descriptor each keeps the queue clear for early exps
        w16 = {}
        for name, src in (("q", wq), ("k", wk), ("v", wv)):
            w16[name] = const.tile(
                [128, CB, H], dtype, tag=f"w{name}", name=f"w16{name}"
            )
            nc.scalar.dma_start(
                w16[name][:], src[:].rearrange("(cb ci) h -> ci cb h", ci=128)
            )
        # mask16[jl, ql] = 1 if ql >= jl else 0 (transposed-score layout)
        mask16 = const.tile([128, 128], dtype, tag="mask")
        nc.gpsimd.memset(mask16[:], 1.0)
        nc.gpsimd.affine_select(
            out=mask16[:], in_=mask16[:],
            compare_op=mybir.AluOpType.is_ge,
            fill=0.0, base=0, pattern=[[1, 128]], channel_multiplier=-1,
        )

        qT16 = persist.tile([128, T], dtype, tag="qT")
        kT16 = persist.tile([128, T], dtype, tag="kT")
        v16 = persist.tile([128, TB, H + 1], dtype, tag="v")
        nc.gpsimd.memset(v16[:, :, H : H + 1], 1.0)  # ones column -> denominators

        for c in range(NCH):
            t0 = c * 512
            # ---- phase A: load + project ----
            x16 = xsb.tile([128, CB, 512], dtype, tag="x16", name=f"x16_c{c}")
            if c == 0:
                # two halves so the first projection matmuls start early
                for h0 in (0, 4):
                    nc.sync.dma_start(
                        x16[:, h0 : h0 + 4, :],
                        xt[h0 * 128 : (h0 + 4) * 128, t0 : t0 + 512].rearrange(
                            "(cb ci) t -> ci cb t", ci=128
                        ),
                    )
            else:
                nc.sync.dma_start(
                    x16[:],
                    xt[:, t0 : t0 + 512].rearrange("(cb ci) t -> ci cb t", ci=128),
                )
            pja = pp.tile([128, 1024], F32, tag="sp", name=f"pj_{c}")
            for name, half in (("q", 0), ("k", 1)):
                for cb in range(CB):
                    nc.tensor.matmul(
                        pja[:, half * 512 : half * 512 + 512],
                        w16[name][:, cb, :], x16[:, cb, :],
                        start=(cb == 0), stop=(cb == CB - 1),
                    )
            nc.vector.tensor_copy(qT16[:, t0 : t0 + 512], pja[:, 0:512])
            nc.vector.tensor_copy(kT16[:, t0 : t0 + 512], pja[:, 512:1024])
            for tb in range(4):
                for cb in range(CB):
                    nc.tensor.matmul(
                        pja[:, tb * 128 : (tb + 1) * 128],
                        x16[:, cb, ts(tb, 128)], w16["v"][:, cb, :],
                        start=(cb == 0), stop=(cb == CB - 1),
                    )
            for tb in range(4):
                nc.vector.tensor_copy(
                    v16[:, c * 4 + tb, 0:H], pja[:, tb * 128 : (tb + 1) * 128]
                )

            # ---- phase B: attention for queries [t0, t0+512) ----
            ops = [
                po.tile([128, 2, 256], F32, tag=f"o{g}", name=f"op_{c}_{g}")
                for g in range(2)
            ]
            njb = 4 * c + 4
            p16s = []
            for p in range(njb // 2):
                sp = pp.tile([128, 1024], F32, tag="sp", name=f"sp_{c}_{p}")
                for ji, off in ((2 * p, 0), (2 * p + 1, 512)):
                    d = ji - 4 * c
                    q_lo = max(d, 0) * 128  # cols left of diagonal: skipped
                    nc.tensor.matmul(
                        sp[:, off + q_lo : off + 512],
                        kT16[:, ts(ji, 128)],
                        qT16[:, t0 + q_lo : t0 + 512],
                        start=True, stop=True,
                    )
                p16 = pP.tile([128, 1024], dtype, tag="p", name=f"p16_{c}_{p}")
                p16s.append(p16)
                nc.scalar.activation(
                    p16[:], sp[:],
                    mybir.ActivationFunctionType.Exp, scale=scale,
                )
                for ji, off in ((2 * p, 0), (2 * p + 1, 512)):
                    d = ji - 4 * c
                    if d >= 1:
                        # overwrite exp(junk) left of the diagonal block
                        nc.vector.memset(p16[:, off : off + d * 128], 0.0)
                    if d >= 0:
                        nc.vector.tensor_mul(
                            p16[:, off + d * 128 : off + (d + 1) * 128],
                            p16[:, off + d * 128 : off + (d + 1) * 128],
                            mask16[:],
                        )
            # AV per query block, each accumulation group uninterleaved:
            # groups sharing a PSUM bank must run sequentially because a
            # group-start clears has_written for the WHOLE bank. Normalize
            # each bank's two query blocks as soon as the bank is done so
            # the epilogue overlaps the other bank's accumulation.
            for qb in range(4):
                i_q = 4 * c + qb
                for ji in range(i_q + 1):
                    off = (ji % 2) * 512
                    nc.tensor.matmul(
                        ops[qb // 2][:, qb % 2, 0 : H + 1],
                        p16s[ji // 2][:, off + qb * 128 : off + (qb + 1) * 128],
                        v16[:, ji, :],
                        start=(ji == 0), stop=(ji == i_q),
                    )
                if qb % 2 == 1:
                    for q2 in (qb - 1, qb):
                        sl = ops[q2 // 2][:, q2 % 2, :]
                        rec = osb.tile([128, 1], F32, tag="rec")
                        nc.vector.reciprocal(rec[:], sl[:, H : H + 1])
                        o32 = osb.tile([128, H], F32, tag="o32")
                        nc.vector.tensor_scalar_mul(o32[:], sl[:, 0:H], rec[:])
                        nc.sync.dma_start(
                            out[t0 + q2 * 128 : t0 + (q2 + 1) * 128, :], o32[:]
                        )

    return nc


_NC_CACHE = None


def _get_nc():
    global _NC_CACHE
    if _NC_CACHE is None:
        _apply_tile_patch()
        _NC_CACHE = build_attention()
    return _NC_CACHE


def kernel(x, Wk, Wq, Wv, trace=False):
    """Full inputs in, full output out. Shards batch across the 8 cores."""
    from concourse.bass_utils import run_bass_kernel_spmd

    x = np.asarray(x, dtype=np.float32)
    Wk16 = np.ascontiguousarray(np.asarray(Wk, dtype=np.float32).astype(np.float16))
    Wq16 = np.ascontiguousarray(np.asarray(Wq, dtype=np.float32).astype(np.float16))
    Wv16 = np.ascontiguousarray(np.asarray(Wv, dtype=np.float32).astype(np.float16))
    assert x.shape == (B, T, C), x.shape
    # host-side layout: contraction dim onto partitions, cast to fp16
    xT16 = np.ascontiguousarray(x.transpose(0, 2, 1).astype(np.float16))

    nc = _get_nc()
    in_maps = [
        {"xt": xT16[b], "wq": Wq16, "wk": Wk16, "wv": Wv16} for b in range(B)
    ]
    res = run_bass_kernel_spmd(nc, in_maps, core_ids=list(range(B)), trace=trace)
    outp = np.stack([res.results[b]["out"] for b in range(B)], axis=0)
    if trace:
        global _LAST_RES
        _LAST_RES = res
        return outp, res.exec_time_ns
    return outp


# revision 19
# speedup vs baseline: 1.0078x; 1.0078x over previous
"""Causal single-head attention (B=8, T=4096, C=1024, H=128) on 8 TRN2
NeuronCores, data-parallel over batch: core b computes batch element b.

Host pre-transposes and pre-casts: each core gets xT [C, T] fp16 (so the
contraction dim is already on partitions -- no on-chip transposes) plus
Wq/Wk/Wv [C, H] fp16 replicated. Output is [T, H] f32.

Per chunk of 512 queries (fully unrolled Tile program, fp16 compute /
fp32 psum):
  phase A: one DMA brings the xT chunk in; PE projects qT/kT [H, 512]
  (contraction C on partitions, N=512 streams) and v [t, H] blocks.
  phase B: key blocks are processed in PAIRS -- two N=512 score matmuls
  land side by side in one 2-bank PSUM tile [128, 1024] and a single
  ScalarE exp (scale fused, fp16 out) covers both, halving the per-call
  ACTIVATE overhead that bottlenecked the per-j version. Diagonal blocks
  compute only the causally needed score columns; the junk exp() left of
  the diagonal is overwritten by a DVE memset, and the diagonal block is
  masked by a triangular multiply. AV accumulates out[q, H+1] over key
  blocks (ones column of v yields softmax denominators); DVE normalizes.
"""
import numpy as np

import concourse.bass as bass
import concourse.mybir as mybir
import concourse.tile as tile
from concourse.bass import ts
from contextlib import ExitStack

F16 = mybir.dt.float16
F32 = mybir.dt.float32

B, T, C, H = 8, 4096, 1024, 128

# ---------------------------------------------------------------------------
# Workaround for the walrus build in this container: each TPB instruction may
# carry at most ONE sync-wait ("Too many sync wait commands" otherwise), but
# Tile attaches several. Keep only the last wait per instruction and hoist the
# others onto preceding same-engine NoOps (engines execute their stream in
# order, so the gating semantics are identical). The tail drain gets the same
# treatment.
# ---------------------------------------------------------------------------
_MAX_WAITS = 1
_orig_add_instruction = tile.TileContext._add_instruction


def _split_waits_add_instruction(self, inst):
    si = inst.sync_info
    if (
        si is not None
        and len(si.on_wait) > _MAX_WAITS
        and inst.engine != mybir.EngineType.Unassigned
    ):
        waits = list(si.on_wait)
        extra, keep = waits[:-_MAX_WAITS], waits[-_MAX_WAITS:]
        for w in extra:
            nop = mybir.InstNoOp(
                name=self.nc.get_next_instruction_name(),
                engine=inst.engine,
                ins=[],
                outs=[],
                bass_nofuse=True,
                sync_info=mybir.SyncInfo(on_wait=[w], on_update=[]),
                debug=inst.debug,
            )
            _orig_add_instruction(self, nop)
        inst.sync_info = mybir.SyncInfo(on_wait=keep, on_update=list(si.on_update))
    return _orig_add_instruction(self, inst)


def _split_drain_and_barrier(self, tick_clock, wait_clock):
    nc = self.nc
    probe = nc.sync.nop(nofuse=True, hint="tile_drain_wait_split")
    wait_clock.add_sem_waits(
        probe.ins, tile.ScopedClock({None: tick_clock.global_clock})
    )
    si = probe.ins.sync_info
    waits = list(si.on_wait) if si is not None else []
    if len(waits) > _MAX_WAITS:
        probe.ins.sync_info = mybir.SyncInfo(
            on_wait=waits[:_MAX_WAITS], on_update=list(si.on_update)
        )
        rest = waits[_MAX_WAITS:]
        for i in range(0, len(rest), _MAX_WAITS):
            extra = nc.sync.nop(nofuse=True, hint=f"tile_drain_wait_split_{i}")
            extra.ins.sync_info = mybir.SyncInfo(
                on_wait=rest[i : i + _MAX_WAITS], on_update=[]
            )
    nc.sync.drain()
    nc.all_engine_barrier()
    assert self.sems is not None
    popped = nc._tile_sem_poison_stack.pop()
    assert popped is self._sem_poison
    nc.clear_and_free_semaphores(list(self.sems.allocated().values()))
    nc.all_engine_barrier()


def _apply_tile_patch():
    tile.TileContext._drain_and_barrier = _split_drain_and_barrier
    tile.TileContext._add_instruction = _split_waits_add_instruction


# ---------------------------------------------------------------------------
# Kernel builder
# ---------------------------------------------------------------------------
def build_attention(dtype=F16):
    TB = T // 128
    CB = C // 128
    NCH = T // 512
    scale = float(H) ** -0.5

    nc = bass.Bass()
    # host pre-arranged layouts: every DMA line is contiguous per partition
    # xt[c_chunk, ci, cb*512 + t] = x[t_global, cb*128 + ci]
    xt = nc.dram_tensor("xt", [NCH, 128, CB * 512], F16, kind="ExternalInput")
    # w*[ci, cb*H + h] = W[cb*128 + ci, h]
    wq = nc.dram_tensor("wq", [128, CB * H], F16, kind="ExternalInput")
    wk = nc.dram_tensor("wk", [128, CB * H], F16, kind="ExternalInput")
    wv = nc.dram_tensor("wv", [128, CB * H], F16, kind="ExternalInput")
    out = nc.dram_tensor("out", [T, H], F32, kind="ExternalOutput")

    with tile.TileContext(nc) as tc, ExitStack() as ctx:
        const = ctx.enter_context(tc.tile_pool(name="const", bufs=1))
        xsb = ctx.enter_context(tc.tile_pool(name="xsb", bufs=3))
        persist = ctx.enter_context(tc.tile_pool(name="persist", bufs=1))
        pP = ctx.enter_context(tc.tile_pool(name="pP", bufs=18))
        osb = ctx.enter_context(tc.tile_pool(name="osb", bufs=4))
        # scores AND projections share one 3-deep pool of 2-bank tiles;
        # ops take the remaining 2 banks (8 total)
        pp = ctx.enter_context(tc.tile_pool(name="pp", bufs=3, space="PSUM"))
        po = ctx.enter_context(tc.tile_pool(name="po", bufs=1, space="PSUM"))

        # weights (replicated, fp16 from host) on the idle Scalar HWDGE
        # queue: fully contiguous transfers
        w16 = {}
        for name, src in (("q", wq), ("k", wk), ("v", wv)):
            w16[name] = const.tile(
                [128, CB, H], dtype, tag=f"w{name}", name=f"w16{name}"
            )
            nc.scalar.dma_start(
                w16[name][:], src[:].rearrange("ci (cb h) -> ci cb h", cb=CB)
            )
        # mask16[jl, ql] = 1 if ql >= jl else 0 (transposed-score layout)
        mask16 = const.tile([128, 128], dtype, tag="mask")
        nc.gpsimd.memset(mask16[:], 1.0)
        nc.gpsimd.affine_select(
            out=mask16[:], in_=mask16[:],
            compare_op=mybir.AluOpType.is_ge,
            fill=0.0, base=0, pattern=[[1, 128]], channel_multiplier=-1,
        )

        qT16 = persist.tile([128, T], dtype, tag="qT")
        kT16 = persist.tile([128, T], dtype, tag="kT")
        v16 = persist.tile([128, TB, H + 1], dtype, tag="v")
        nc.gpsimd.memset(v16[:, :, H : H + 1], 1.0)  # ones column -> denominators

        for c in range(NCH):
            t0 = c * 512
            # ---- phase A: load + project ----
            x16 = xsb.tile([128, CB, 512], dtype, tag="x16", name=f"x16_c{c}")
            if c == 0:
                # two halves so the first projection matmuls start early
                for h0 in (0, 4):
                    nc.sync.dma_start(
                        x16[:, h0 : h0 + 4, :],
                        xt[c, :, h0 * 512 : (h0 + 4) * 512].rearrange(
                            "ci (cb t) -> ci cb t", cb=4
                        ),
                    )
            else:
                nc.sync.dma_start(
                    x16[:],
                    xt[c, :, :].rearrange("ci (cb t) -> ci cb t", cb=CB),
                )
            pja = pp.tile([128, 1024], F32, tag="sp", name=f"pj_{c}")
            for name, half in (("q", 0), ("k", 1)):
                for cb in range(CB):
                    nc.tensor.matmul(
                        pja[:, half * 512 : half * 512 + 512],
                        w16[name][:, cb, :], x16[:, cb, :],
                        start=(cb == 0), stop=(cb == CB - 1),
                    )
            nc.vector.tensor_copy(qT16[:, t0 : t0 + 512], pja[:, 0:512])
            nc.vector.tensor_copy(kT16[:, t0 : t0 + 512], pja[:, 512:1024])
            for tb in range(4):
                for cb in range(CB):
                    nc.tensor.matmul(
                        pja[:, tb * 128 : (tb + 1) * 128],
                        x16[:, cb, ts(tb, 128)], w16["v"][:, cb, :],
                        start=(cb == 0), stop=(cb == CB - 1),
                    )
            for tb in range(4):
                nc.vector.tensor_copy(
                    v16[:, c * 4 + tb, 0:H], pja[:, tb * 128 : (tb + 1) * 128]
                )

            # ---- phase B: attention for queries [t0, t0+512) ----
            ops = [
                po.tile([128, 2, 256], F32, tag=f"o{g}", name=f"op_{c}_{g}")
                for g in range(2)
            ]
            njb = 4 * c + 4
            p16s = []
            for p in range(njb // 2):
                sp = pp.tile([128, 1024], F32, tag="sp", name=f"sp_{c}_{p}")
                for ji, off in ((2 * p, 0), (2 * p + 1, 512)):
                    d = ji - 4 * c
                    q_lo = max(d, 0) * 128  # cols left of diagonal: skipped
                    nc.tensor.matmul(
                        sp[:, off + q_lo : off + 512],
                        kT16[:, ts(ji, 128)],
                        qT16[:, t0 + q_lo : t0 + 512],
                        start=True, stop=True,
                    )
                p16 = pP.tile([128, 1024], dtype, tag="p", name=f"p16_{c}_{p}")
                p16s.append(p16)
                nc.scalar.activation(
                    p16[:], sp[:],
                    mybir.ActivationFunctionType.Exp, scale=scale,
                )
                for ji, off in ((2 * p, 0), (2 * p + 1, 512)):
                    d = ji - 4 * c
                    if d >= 1:
                        # overwrite exp(junk) left of the diagonal block
                        nc.vector.memset(p16[:, off : off + d * 128], 0.0)
                    if d >= 0:
                        nc.vector.tensor_mul(
                            p16[:, off + d * 128 : off + (d + 1) * 128],
                            p16[:, off + d * 128 : off + (d + 1) * 128],
                            mask16[:],
                        )
            # AV per query block, each accumulation group uninterleaved:
            # groups sharing a PSUM bank must run sequentially because a
            # group-start clears has_written for the WHOLE bank. Normalize
            # each bank's two query blocks as soon as the bank is done so
            # the epilogue overlaps the other bank's accumulation.
            for qb in range(4):
                i_q = 4 * c + qb
                for ji in range(i_q + 1):
                    off = (ji % 2) * 512
                    nc.tensor.matmul(
                        ops[qb // 2][:, qb % 2, 0 : H + 1],
                        p16s[ji // 2][:, off + qb * 128 : off + (qb + 1) * 128],
                        v16[:, ji, :],
                        start=(ji == 0), stop=(ji == i_q),
                    )
                if qb % 2 == 1:
                    for q2 in (qb - 1, qb):
                        sl = ops[q2 // 2][:, q2 % 2, :]
                        rec = osb.tile([128, 1], F32, tag="rec")
                        nc.vector.reciprocal(rec[:], sl[:, H : H + 1])
                        o32 = osb.tile([128, H], F32, tag="o32")
                        nc.vector.tensor_scalar_mul(o32[:], sl[:, 0:H], rec[:])
                        nc.sync.dma_start(
                            out[t0 + q2 * 128 : t0 + (q2 + 1) * 128, :], o32[:]
                        )

    return nc


_NC_CACHE = None


def _get_nc():
    global _NC_CACHE
    if _NC_CACHE is None:
        _apply_tile_patch()
        _NC_CACHE = build_attention()
    return _NC_CACHE


def kernel(x, Wk, Wq, Wv, trace=False):
    """Full inputs in, full output out. Shards batch across the 8 cores."""
    from concourse.bass_utils import run_bass_kernel_spmd

    x = np.asarray(x, dtype=np.float32)
    assert x.shape == (B, T, C), x.shape

    def _warr(w):
        # [C, H] f32 -> [ci, cb*H] fp16 so the on-chip tile loads contiguously
        w16 = np.asarray(w, dtype=np.float32).astype(np.float16)
        return np.ascontiguousarray(
            w16.reshape(C // 128, 128, H).transpose(1, 0, 2).reshape(128, -1)
        )

    Wk16, Wq16, Wv16 = _warr(Wk), _warr(Wq), _warr(Wv)
    # [B,T,C] -> xt[b, chunk, ci, cb*512+t] = x[b, chunk*512+t, cb*128+ci]
    xT16 = np.ascontiguousarray(
        x.transpose(0, 2, 1)
        .astype(np.float16)
        .reshape(B, C // 128, 128, T // 512, 512)
        .transpose(0, 3, 2, 1, 4)
        .reshape(B, T // 512, 128, -1)
    )

    nc = _get_nc()
    in_maps = [
        {"xt": xT16[b], "wq": Wq16, "wk": Wk16, "wv": Wv16} for b in range(B)
    ]
    res = run_bass_kernel_spmd(nc, in_maps, core_ids=list(range(B)), trace=trace)
    outp = np.stack([res.results[b]["out"] for b in range(B)], axis=0)
    if trace:
        global _LAST_RES
        _LAST_RES = res
        return outp, res.exec_time_ns
    return outp
